# revision 2
# baseline (speedup 1.0000x reference)
"""PASA group-softmax high-pass downsample kernel for 8 Trainium2 NeuronCores.

Reference computation (n=4, c=64, h=w=128, G=2 groups, K=3, stride 2):
  xp     = reflect_pad(x, 1)
  sigma  = conv3x3(xp, conv_w)                    # [n, 18, h, w]
  sigma  = sigma * bn_scale + bn_shift            # BN (inference)
  sigma  = softmax(sigma, axis=1)                 # over all 18 channels
  sigma  = onehot(center) - sigma                 # high-pass
  out[n,g,c,i,j] = sum_k patches[n,g,c,k,i,j] * sigma[n,g,k,i,j]
  return out[:, :, ::2, ::2]                      # [4, 64, 64, 64]

Layout (per core = one image n, one h-half; sub-halves A/B stacked on
partitions 0-63 / 64-127 so all engines run 128 lanes wide):

  x slab  [128, 66, 66] bf16 "phase planes": the padded input rows for the
          sub-half, split by (row, col) parity into 4 planes
          p00/p01/p10/p11 at plane-row bases 0/17/34/50, each row padded to
          66 cols.  Every stride-2 patch view of the original conv becomes a
          UNIT-STRIDE view of one plane: tap (dy,dx) -> plane(dy%2, dx%2)
          rows +dy//2, cols +dx//2.  Plane p11 row 0.. IS the center pixel
          grid (xc) - no separate center DMA.
  conv    18 matmuls (9 taps x 2 row-chunks), contraction 128 (both halves),
          outputs 36 sigma rows; chunk0 -> psum parts 0:64 (36:64 zeroed via
          zero weight cols), chunk1 -> parts 64:100 (col-tile 64).
  exp     one ACT op [100, 512] with per-partition BN-shift bias -> e bf16.
  D       2 matmuls (per chunk) with a selector; reciprocal_approx_fast on
          DVE; cast bf16 on ACT; broadcast r back via one bf16 matmul;
          f = e * rbig (normalized softmax weights).
  apply   per tap: ebig = (-esel_k) @ f broadcasts -F to channel layout
          (2 matmuls); prod = patch * ebig on DVE; PSUM-accumulate
          y = xc - sum taps via identity matmuls (the chain STARTS with
          ident @ xc, esel is negated host-side).
  store   ACT copies acc PSUM -> SBUF f32, DMA out.
"""

import os
import ml_dtypes
import numpy as np

import concourse.bass as bass
import concourse.tile as tile
from concourse import bacc, mybir
from concourse.bass_utils import run_bass_kernel_spmd

F32 = mybir.dt.float32
BF16 = mybir.dt.bfloat16

N, C, H, W = 4, 64, 128, 128
G, K = 2, 3
K2 = K * K
EPS = 1e-5
NCORES = 8
HO, WO = H // 2, W // 2            # 64, 64 output spatial
RS = 16                            # output rows per sub-half (A/B)
CH_ROWS = RS // 2                  # 8 output rows per chunk
CHUNK = CH_ROWS * WO               # 512 positions per chunk
POS = RS * WO                      # 1024 positions per sub-half/partition

# phase-plane row bases inside the [128, 66, 66] slab
PB = {(0, 0): 0, (0, 1): 17, (1, 0): 34, (1, 1): 50}
PLANE_ROWS = {(0, 0): 17, (0, 1): 17, (1, 0): 16, (1, 1): 16}
SLAB_R, SLAB_C = 66, 66

NWARM = 6

_compiled = None


def _tap_view(x_sb, k, rows, r0=0):
    """Unit-stride patch view for tap k: [128, rows, 64] at chunk row r0."""
    dy, dx = k // K, k % K
    base = PB[(dy % 2, dx % 2)] + dy // 2
    c0 = dx // 2
    return x_sb[:, base + r0 : base + r0 + rows, c0 : c0 + 64]


def _build_program():
    nc = bacc.Bacc(
        "TRN2", target_bir_lowering=False, debug=False, num_devices=NCORES
    )

    xab = nc.dram_tensor("xab", [128, SLAB_R, SLAB_C], BF16, kind="ExternalInput")
    wts = nc.dram_tensor("wts", [128, K2, 64], BF16, kind="ExternalInput")
    bias = nc.dram_tensor("bias", [128, 1], F32, kind="ExternalInput")
    sel = nc.dram_tensor("sel", [128, 34], BF16, kind="ExternalInput")
    rsel = nc.dram_tensor("rsel", [34, 128], BF16, kind="ExternalInput")
    esel = nc.dram_tensor("esel", [72, K2, 128], BF16, kind="ExternalInput")
    ident = nc.dram_tensor("ident", [128, 128], BF16, kind="ExternalInput")
    y = nc.dram_tensor("y", [128, RS, WO], F32, kind="ExternalOutput")
    warm_out = nc.dram_tensor("warm_out", [1, 2], F32, kind="ExternalOutput")

    with tile.TileContext(nc) as tc:
        with (
            tc.tile_pool(name="singles", bufs=1) as singles,
            tc.tile_pool(name="psum", bufs=1, space="PSUM") as psum,
            tc.tile_pool(name="ebig", bufs=2, space="PSUM") as ebig_pool,
            tc.tile_pool(name="work", bufs=3) as work,
        ):
            # ---- small constant loads on the sync ring ----
            ident_sb = singles.tile([128, 128], BF16)
            nc.sync.dma_start(ident_sb[:], ident.ap())
            w_sb = singles.tile([128, K2, 64], BF16)
            nc.sync.dma_start(w_sb[:], wts.ap())
            bias_sb = singles.tile([128, 1], F32)
            nc.sync.dma_start(bias_sb[:], bias.ap())
            sel_sb = singles.tile([128, 34], BF16)
            nc.sync.dma_start(sel_sb[:], sel.ap())
            rsel_sb = singles.tile([34, 128], BF16)
            nc.sync.dma_start(rsel_sb[:], rsel.ap())
            esel_sb = singles.tile([128, K2, 128], BF16)
            nc.sync.dma_start(esel_sb[0:36], esel.ap()[0:36])
            nc.sync.dma_start(esel_sb[64:100], esel.ap()[36:72])

            # prewarm ACT's exp table (dep-free) so the ~1.3us table load
            # stays off the critical path
            warm_in = work.tile([1, 1], F32, tag="warm_in")
            nc.gpsimd.memset(warm_in[:], 0.25)
            warm_e = work.tile([1, 1], F32, tag="warm")
            nc.scalar.activation(warm_e[:], warm_in[:],
                                 mybir.ActivationFunctionType.Exp)

            # ---- x slab: chunk0 plane-halves first, on two DMA rings ----
            x_sb = singles.tile([128, SLAB_R, SLAB_C], BF16)
            ranges_a = [(0, 9), (17, 26), (34, 42), (50, 58)]
            ranges_b = [(9, 17), (26, 34), (42, 50), (58, 66)]
            for i, (r0, r1) in enumerate(ranges_a + ranges_b):
                eng = nc.gpsimd if i % 2 == 0 else nc.scalar
                eng.dma_start(x_sb[:, r0:r1], xab.ap()[:, r0:r1])

            # PE warm-up junk matmuls ramp the clock before conv
            warm_ps = psum.tile([128, 128], F32, tag="dr",
                                padded_shape=[128, CHUNK])
            for i in range(NWARM):
                nc.tensor.matmul(warm_ps[:], ident_sb[:], ident_sb[:],
                                 start=(i == 0), stop=(i == NWARM - 1),
                                 skip_group_check=True)
            warm_sb = work.tile([1, 2], F32, tag="warm_sb")
            nc.vector.tensor_copy(warm_sb[:], warm_ps[0:1, 0:2])
            nc.sync.dma_start(warm_out.ap(), warm_sb[:])

            # ---- conv: 2 chunks x 9 taps, contraction over both halves ----
            sigma_ps = psum.tile([128, CHUNK], F32, tag="acc",
                                 padded_shape=[128, POS])
            for ch in range(2):
                wlim = 64 if ch == 0 else 36
                for k in range(K2):
                    nc.tensor.matmul(
                        sigma_ps[64 * ch : 64 * ch + wlim, :],
                        w_sb[:, k, 0:wlim],
                        _tap_view(x_sb, k, CH_ROWS, r0=CH_ROWS * ch),
                        start=(k == 0),
                        stop=(k == K2 - 1),
                        tile_position=(0, 64 * ch),
                        skip_group_check=True,
                    )

            # ---- E = exp(sigma + bn_shift) in bf16, one ACT op ----
            e_sb = singles.tile([128, CHUNK], BF16)
            nc.scalar.activation(
                e_sb[0:100], sigma_ps[0:100], mybir.ActivationFunctionType.Exp,
                bias=bias_sb[0:100], scale=1.0,
            )

            # ---- denominator rows (ch0 -> d[0:32] dup-filled, ch1 -> d[32:34])
            d_ps = psum.tile([64, CHUNK], F32, tag="dr",
                             padded_shape=[128, CHUNK])
            nc.tensor.matmul(d_ps[0:32, :], sel_sb[0:36, 0:32], e_sb[0:36, :],
                             tile_position=(0, 0), skip_group_check=True)
            nc.tensor.matmul(d_ps[32:34, :], sel_sb[64:100, 32:34],
                             e_sb[64:100, :],
                             tile_position=(64, 32), skip_group_check=True)

            # ---- r = 1/D (fast approx), cast bf16 on ACT, broadcast ----
            r_sb = singles.tile([34, CHUNK], F32)
            nc.vector.reciprocal_approx_fast(r_sb[:], d_ps[0:34, :])
            r_bf = singles.tile([34, CHUNK], BF16)
            nc.scalar.copy(r_bf[:], r_sb[:])
            rbig_ps = psum.tile([128, CHUNK], F32, tag="dr")
            nc.tensor.matmul(rbig_ps[:], rsel_sb[:], r_bf[:])

            # ---- normalized weights f = e * rbig, bf16 ----
            f_sb = singles.tile([128, CHUNK], BF16)
            nc.vector.tensor_mul(f_sb[0:100], e_sb[0:100], rbig_ps[0:100])

            # ---- apply: acc = xc - sum_k patch_k * F_k (esel negated) ----
            acc_ps = psum.tile([128, POS], F32, tag="acc")
            for ch in range(2):
                nc.tensor.matmul(
                    acc_ps[:, CHUNK * ch : CHUNK * (ch + 1)],
                    ident_sb[:],
                    _tap_view(x_sb, 4, CH_ROWS, r0=CH_ROWS * ch),  # center
                    start=True, stop=False, skip_group_check=True,
                )
            prods = []
            for k in range(K2):
                ebig = ebig_pool.tile([128, POS], F32, name=f"ebig{k}",
                                      tag="ebig")
                for ch in range(2):
                    nc.tensor.matmul(
                        ebig[:, CHUNK * ch : CHUNK * (ch + 1)],
                        esel_sb[64 * ch : 64 * ch + 36, k, :],
                        f_sb[64 * ch : 64 * ch + 36, :],
                        tile_position=(64 * ch, 0),
                        skip_group_check=True,
                    )
                prod = work.tile([128, POS], BF16, name=f"prod{k}", tag="prod")
                nc.vector.tensor_mul(
                    prod[:].rearrange("p (r c) -> p r c", r=RS),
                    _tap_view(x_sb, k, RS),
                    ebig[:].rearrange("p (r c) -> p r c", r=RS),
                )
                prods.append(prod)
                if k >= 1:
                    for ch in range(2):
                        nc.tensor.matmul(
                            acc_ps[:, CHUNK * ch : CHUNK * (ch + 1)],
                            ident_sb[:],
                            prods[k - 1][:, CHUNK * ch : CHUNK * (ch + 1)],
                            start=False, stop=False, skip_group_check=True,
                        )
            for ch in range(2):
                nc.tensor.matmul(
                    acc_ps[:, CHUNK * ch : CHUNK * (ch + 1)],
                    ident_sb[:],
                    prods[K2 - 1][:, CHUNK * ch : CHUNK * (ch + 1)],
                    start=False, stop=True, skip_group_check=True,
                )

            # ---- store: ACT copies PSUM -> SBUF f32, overlapped DMAs ----
            y_sb = work.tile([128, RS, WO], F32)
            acc3 = acc_ps[:].rearrange("p (r c) -> p r c", r=RS)
            for ch in range(2):
                rr = slice(CH_ROWS * ch, CH_ROWS * (ch + 1))
                nc.scalar.copy(y_sb[:, rr], acc3[:, rr])
                eng = nc.sync if ch == 0 else nc.gpsimd
                eng.dma_start(y.ap()[:, rr], y_sb[:, rr])

    nc.compile()
    return nc


def _host_inputs(x, conv_w, gamma, beta, running_mean, running_var):
    """Prepare per-core input dicts (sharding + BN folding + phase planes)."""
    scale = gamma / np.sqrt(running_var + EPS)
    shift = beta - running_mean * scale

    # conv weights: lhsT [128, 9, 64]; block-diag over sub-halves:
    # partitions 0:64 (A data) -> outs 0:18, partitions 64:128 (B) -> 18:36
    w_scaled = conv_w * scale[:, None, None, None]            # [18, 64, 3, 3]
    wts = np.zeros((128, K2, 64), np.float32)
    for k in range(K2):
        dy, dx = k // K, k % K
        wl = w_scaled[:, :, dy, dx].T                         # [64c, 18o]
        wts[0:64, k, 0:18] = wl
        wts[64:128, k, 18:36] = wl
    wts = wts.astype(ml_dtypes.bfloat16)

    # exp bias rows: compact layout p = 18*half + (g*9+k)
    bias = np.zeros((128, 1), np.float32)
    bias[0:18, 0] = shift
    bias[18:36, 0] = shift
    bias[64:82, 0] = shift
    bias[82:100, 0] = shift

    # D selector: ch0 -> d rows 0/1 (cols 2:32 dup col0 to keep recip finite),
    # ch1 -> d rows 32/33
    sel = np.zeros((128, 34), np.float32)
    sel[0:18, 0] = 1.0
    sel[18:36, 1] = 1.0
    sel[0:18, 2:32] = 1.0
    sel[64:82, 32] = 1.0
    sel[82:100, 33] = 1.0
    sel = sel.astype(ml_dtypes.bfloat16)

    # r broadcast: d row -> compact partitions
    rsel = np.zeros((34, 128), np.float32)
    rsel[0, 0:18] = 1.0
    rsel[1, 18:36] = 1.0
    rsel[32, 64:82] = 1.0
    rsel[33, 82:100] = 1.0
    rsel = rsel.astype(ml_dtypes.bfloat16)

    # NEGATED tap selector: rows 0:36 for ch0 (f rows 0:36), rows 36:72 are
    # the same matrix loaded at tile partitions 64:100 for ch1.
    # channel partition j: half h=j//64, group g=(j%64)//32 -> f row
    # 18*h + 9*g + k  (within the chunk's 36-row block)
    esel = np.zeros((72, K2, 128), np.float32)
    for k in range(K2):
        for j in range(128):
            r = 18 * (j // 64) + 9 * ((j % 64) // 32) + k
            esel[r, k, j] = -1.0
            esel[36 + r, k, j] = -1.0
    esel = esel.astype(ml_dtypes.bfloat16)

    ident = np.eye(128, dtype=np.float32).astype(ml_dtypes.bfloat16)

    xpad = np.pad(x, ((0, 0), (0, 0), (1, 1), (1, 1)), mode="reflect")

    in_maps = []
    for core in range(NCORES):
        n, h = core // 2, core % 2
        r0 = 64 * h
        xa = np.zeros((128, SLAB_R, SLAB_C), np.float32)
        for half, rs in ((0, r0), (1, r0 + 32)):
            slab = xpad[n, :, rs : rs + 33, :]                # [64, 33, 130]
            p = slice(64 * half, 64 * half + 64)
            xa[p, 0:17, 0:65] = slab[:, 0::2, 0::2]
            xa[p, 17:34, 0:65] = slab[:, 0::2, 1::2]
            xa[p, 34:50, 0:65] = slab[:, 1::2, 0::2]
            xa[p, 50:66, 0:65] = slab[:, 1::2, 1::2]
        in_maps.append(
            {"xab": xa.astype(ml_dtypes.bfloat16), "wts": wts, "bias": bias,
             "sel": sel, "rsel": rsel, "esel": esel, "ident": ident}
        )
    return in_maps


def _gather_output(results):
    out = np.empty((N, C, HO, WO), np.float32)
    for core, res in enumerate(results):
        n, h = core // 2, core % 2
        ycore = res["y"].reshape(2, C, RS, WO)
        out[n, :, 32 * h : 32 * h + RS, :] = ycore[0]
        out[n, :, 32 * h + RS : 32 * h + 2 * RS, :] = ycore[1]
    return out


def _ensure_ntff_hook():
    """Install the axon NTFF profile hook if the image's antenv lacks it."""
    try:
        from antenv import axon_hooks  # noqa: F401
        return
    except ImportError:
        pass
    try:
        import sys
        import types

        import antenv
        from trn_agent_boot.trn_boot import _ntff_profile_via_ctypes

        hook = _ntff_profile_via_ctypes("/opt/axon/libaxon_pjrt.so")
        mod = types.ModuleType("antenv.axon_hooks")
        state = {"hook": hook}
        mod.get_axon_ntff_profile_hook = lambda: state["hook"]
        mod.set_axon_ntff_profile_hook = lambda h: state.update(hook=h)
        sys.modules["antenv.axon_hooks"] = mod
        antenv.axon_hooks = mod
    except Exception:
        pass


def kernel(x, conv_w, gamma, beta, running_mean, running_var):
    global _compiled
    x = np.asarray(x, np.float32)
    conv_w = np.asarray(conv_w, np.float32)
    gamma = np.asarray(gamma, np.float32)
    beta = np.asarray(beta, np.float32)
    running_mean = np.asarray(running_mean, np.float32)
    running_var = np.asarray(running_var, np.float32)

    if _compiled is None:
        _compiled = _build_program()
    nc = _compiled

    in_maps = _host_inputs(x, conv_w, gamma, beta, running_mean, running_var)
    trace = bool(int(os.environ.get("PASA_TRACE", "0")))
    if trace:
        _ensure_ntff_hook()
    res = run_bass_kernel_spmd(
        nc, in_maps, core_ids=list(range(NCORES)), trace=trace
    )
    kernel.last_results = res
    return _gather_output(res.results)


if __name__ == "__main__":
    # quick CoreSim check of core 0 against a numpy re-implementation
    from concourse.bass_interp import CoreSim

    rng = np.random.default_rng(0)
    x = rng.standard_normal((N, C, H, W), np.float32)
    conv_w = (rng.standard_normal((G * K2, C, K, K), np.float32)
              * np.sqrt(2.0 / (G * K2 * K * K)))
    gamma = rng.uniform(0.5, 1.5, G * K2).astype(np.float32)
    beta = (rng.standard_normal(G * K2) * 0.1).astype(np.float32)
    rmean = (rng.standard_normal(G * K2) * 0.1).astype(np.float32)
    rvar = rng.uniform(0.5, 1.5, G * K2).astype(np.float32)

    nc = _build_program()
    in_maps = _host_inputs(x, conv_w, gamma, beta, rmean, rvar)
    sim = CoreSim(nc)
    for k, v in in_maps[0].items():
        sim.tensor(k)[:] = v
    sim.simulate(check_with_hw=False)
    ysim = np.array(sim.tensor("y")).reshape(2, C, RS, WO)

    # numpy reference for core 0 region (image 0, output rows 0..32)
    scale = gamma / np.sqrt(rvar + EPS)
    shift = beta - rmean * scale
    xpad = np.pad(x[0], ((0, 0), (1, 1), (1, 1)), mode="reflect")
    sig = np.zeros((G * K2, 32, WO), np.float32)
    for o in range(G * K2):
        for dy in range(K):
            for dx in range(K):
                sig[o] += np.einsum(
                    "crw->rw",
                    conv_w[o, :, dy, dx][:, None, None]
                    * xpad[:, dy : dy + 64 : 2, dx : dx + 128 : 2],
                )
    sig = sig * scale[:, None, None] + shift[:, None, None]
    e = np.exp(sig)
    r = 1.0 / e.sum(0)
    acc = np.zeros((C, 32, WO), np.float32)
    for g in range(G):
        for k in range(K2):
            dy, dx = k // K, k % K
            acc[32 * g : 32 * g + 32] += (
                xpad[32 * g : 32 * g + 32, dy : dy + 64 : 2, dx : dx + 128 : 2]
                * e[g * K2 + k][None]
            )
    ref = (xpad[:, 1:65:2, 1:129:2] - acc * r[None]).astype(np.float32)

    got = np.concatenate([ysim[0], ysim[1]], axis=1)
    err = np.abs(got - ref).max() / np.abs(ref).max()
    print("sim rel err:", err)
